# revision 70
# baseline (speedup 1.0000x reference)
"""Trainium2 Bass kernel for the UR5e reflected-mass cost function.

Closed-form math (per sample n of 131072 = 2048 b x 64 h):
  The last joint (q6) never affects the output (its Jacobian column is 0),
  and in the q1-rotated "cylindrical" frame every frame origin is
  p_i = (A_i, B_i, C_i) with the z-axes {z0=ez, z1=z2=z3=(0,1,0),
  z4=(s234,0,-c234)}.  All Jacobian columns, the 5x5 mass matrix, and the
  end-effector direction reduce to ~260 scalar ops instead of the naive
  ~670 of the frame-by-frame DH chain.

Implementation: every per-sample scalar is a [128,128] f32 SBUF tile
(16384 samples per core, 8 cores data-parallel over b).  The computation
is a symbolic scalar DAG with CSE + constant folding + STT fusion,
scheduled onto the DVE/ACT/GPSIMD engines with an earliest-finish-time
list scheduler and emitted through the Tile framework.
"""

import math
import numpy as np

# ----------------------------------------------------------------------------
# constants
# ----------------------------------------------------------------------------

PI = math.pi
A2C, A3C = -0.425, -0.3922
D1, D4, D5, D6 = 0.1625, 0.1333, 0.0997, 0.0996
# LINK_MASS[i] sits at frame origin p_{i+1}; link 0 (at p1) never moves.
M1, M2, M3, M4, M5 = 8.058, 2.846, 1.37, 1.3, 0.365
M23 = M2 + M3
M45 = M4 + M5
ROTOR = 0.1
MAGIC = 12582912.0  # 1.5 * 2**23 f32 round-to-int trick

# host channel order handed to the device
# 0:q2 1:q3 2:q4 3:q1 4:q5 5:hx 6:hy 7:hz
SRC_COLS = [7, 8, 9, 6, 10, 19, 20, 21]

# ----------------------------------------------------------------------------
# symbolic scalar DAG
# ----------------------------------------------------------------------------


class Expr:
    __slots__ = ("op", "args", "c", "id", "users", "engine", "fused_into",
                 "slot", "order", "prio", "start", "finish", "width",
                 "pack_into")

    def __init__(self, op, args=(), c=None, i=0):
        self.op = op
        self.args = args
        self.c = c
        self.id = i
        self.users = []
        self.engine = None
        self.fused_into = None
        self.slot = None
        self.order = None
        self.prio = 0.0
        self.start = 0.0
        self.finish = 0.0
        self.width = 1
        self.pack_into = None  # (pack_node, slot) for co-located members


class Graph:
    def __init__(self):
        self.nodes = []
        self.cse = {}

    def _mk(self, op, args=(), c=None):
        key = (op, tuple(a.id for a in args), c)
        n = self.cse.get(key)
        if n is None:
            n = Expr(op, args, c, len(self.nodes))
            self.nodes.append(n)
            self.cse[key] = n
        return n

    def C(self, v):
        return self._mk("const", c=float(v))

    def IN(self, ch):
        return self._mk("in", c=ch)

    def add(self, x, y):
        if x.op == "const" and y.op == "const":
            return self.C(x.c + y.c)
        if x.op == "const":
            x, y = y, x
        if y.op == "const":
            if y.c == 0.0:
                return x
            return self._mk("cadd", (x,), y.c)
        a, b = (x, y) if x.id <= y.id else (y, x)
        return self._mk("add", (a, b))

    def sub(self, x, y):
        if x.op == "const" and y.op == "const":
            return self.C(x.c - y.c)
        if y.op == "const":
            if y.c == 0.0:
                return x
            return self._mk("cadd", (x,), -y.c)
        if x.op == "const" and x.c == 0.0:
            return self.cmul(-1.0, y)
        if x is y:
            return self.C(0.0)
        return self._mk("sub", (x, y))

    def cmul(self, c, x):
        c = float(c)
        if x.op == "const":
            return self.C(c * x.c)
        if c == 0.0:
            return self.C(0.0)
        if c == 1.0:
            return x
        if x.op == "cmul":
            return self.cmul(c * x.c, x.args[0])
        return self._mk("cmul", (x,), c)

    def mul(self, x, y):
        if x.op == "const":
            return self.cmul(x.c, y)
        if y.op == "const":
            return self.cmul(y.c, x)
        if x.op == "cmul" and y.op == "cmul":
            return self.cmul(x.c * y.c, self.mul(x.args[0], y.args[0]))
        if x.op == "cmul":
            return self.cmul(x.c, self.mul(x.args[0], y))
        if y.op == "cmul":
            return self.cmul(y.c, self.mul(x, y.args[0]))
        if x is y:
            return self._mk("square", (x,))
        a, b = (x, y) if x.id <= y.id else (y, x)
        return self._mk("mul", (a, b))

    def ts2(self, x, s1, op0, s2, op1):
        return self._mk("ts2", (x,), (float(s1), op0, float(s2), op1))

    def sincos(self, q):
        """(sin q, cos q) sharing one range reduction.
        r0 = q - 2*pi*round(q/2pi) in [-pi, pi]; sin = Sin(r0).
        cos = Sin(r0c + pi/2) where r0c = r0 - 2pi*(r0 >= pi/2), keeping the
        Sin argument in [-pi, pi]."""
        inv2pi = 1.0 / (2.0 * PI)
        t1 = self.ts2(q, inv2pi, "mult", MAGIC, "add")
        k = self._mk("cadd", (t1,), -MAGIC)
        r0 = self.add(self.cmul(-2.0 * PI, k), q)  # fuses to one STT
        s = self._mk("sin", (r0,), (1.0, 0.0))
        ge = self._mk("ts2", (r0,), (PI / 2, "is_ge", 1.0, "mult"))
        r0c = self.add(self.cmul(-2.0 * PI, ge), r0)  # STT
        c = self._mk("sin", (r0c,), (1.0, PI / 2))
        return s, c

    def sqrt_(self, x):
        return self._mk("sqrt", (x,))

    def recip(self, x):
        return self._mk("recip", (x,))

    def sq(self, x):
        return self._mk("square", (x,))

    # ---- wide (width-n) machinery ----
    def pk(self, *members):
        """Co-locate width-1 emitted ops into one [128, n*128] tile.
        Free: members write directly into the pack's tile slots."""
        for m in members:
            assert m.op not in ("const", "in")
        p = self._mk("pack", tuple(members))
        p.width = len(members)
        for i, m in enumerate(members):
            m.pack_into = (p, i)
        return p

    def rev(self, p):
        """Swapped-halves view of a pair (negative-stride AP). Free."""
        n = self._mk("rev", (p,))
        n.width = 2
        return n

    def half(self, p, i):
        """View of one slot of a wide node. Free."""
        n = self._mk("half", (p,), i)
        return n

    def vslice(self, p, lo, w):
        """View of w contiguous slots [lo, lo+w) of a wide node. Free."""
        n = self._mk("vslice", (p,), (lo, w))
        n.width = w
        return n

    def fold(self, p, op, swap=False):
        """[128,128] result = left op right (or right op left) of a pair."""
        return self._mk("fold", (p,), (op, bool(swap)))

    def finalize_widths(self):
        for n in self.nodes:
            if n.op == "pack":
                n.width = len(n.args)
            elif n.op == "rev":
                n.width = 2
            elif n.op == "vslice":
                n.width = n.c[1]
            elif n.op in ("half", "fold"):
                n.width = 1
            elif n.args:
                n.width = max([a.width for a in n.args] + [1])


def build_graph():
    """Returns (graph, cost_neg_node). cost_neg = -cost per sample."""
    g = Graph()
    q2, q3, q4, q1, q5 = (g.IN(i) for i in range(5))
    hx, hy, hz = (g.IN(5 + i) for i in range(3))

    q23 = g.add(q2, q3)
    q234 = g.add(q23, q4)
    s1, c1 = g.sincos(q1)
    s2, c2 = g.sincos(q2)
    s23, c23 = g.sincos(q23)
    s234, c234 = g.sincos(q234)
    s5, c5 = g.sincos(q5)

    # cylindrical coordinates in (A|K) pairs (A1 = K1 = 0, K = C - d1)
    cs2 = g.pk(c2, s2)
    cs23 = g.pk(c23, s23)
    cs234 = g.pk(c234, s234)
    A2K2 = g.cmul(A2C, cs2)                       # [A2|K2] wide ts
    EK3 = g.add(A2K2, g.cmul(A3C, cs23))          # [E|K3] wide STT
    A2 = g.half(A2K2, 0)
    K2 = g.half(A2K2, 1)
    E = g.half(EK3, 0)
    K3 = g.half(EK3, 1)
    ccsc = g.mul(cs234, s5)                       # [cc|sc] broadcast mul
    c45s45 = g.mul(cs234, c5)                     # [c45|s45] broadcast mul
    c45 = g.half(c45s45, 0)
    s45 = g.half(c45s45, 1)
    # A5 = E + d5*s234 ; K5 = K3 - d5*c234  (different signs: packed scalars)
    A5 = g.add(E, g.cmul(D5, s234))
    K5 = g.sub(K3, g.cmul(D5, c234))
    A5K5 = g.pk(A5, K5)
    A6K6 = g.sub(A5K5, g.cmul(D6, ccsc))          # wide STT
    A6 = g.half(A6K6, 0)
    K6 = g.half(A6K6, 1)
    B6 = g.ts2(c5, D6, "mult", D4, "add")         # B6 = d4 + d6*c5

    # squares (wide on ACT)
    sq2 = g.sq(A2K2)
    sq3 = g.sq(EK3)
    sq5 = g.sq(A5K5)
    sq6 = g.sq(A6K6)
    B6s = g.sq(B6)

    # weighted square sums [SA|SK] (suffix style so S45 comes free)
    SS45 = g.add(g.cmul(M5, sq6), g.cmul(M4, sq5))
    SS = g.add(g.add(SS45, g.cmul(M23, sq3)), g.cmul(M1, sq2))
    SA = g.half(SS, 0)
    M11nr = g.fold(SS, "add")
    M11 = g.add(M11nr, g.C(ROTOR))
    M00 = g.add(g.add(SA, g.cmul(M5, B6s)), g.C((M3 + M4) * D4 * D4 + ROTOR))
    S45 = g.fold(SS45, "add")

    # weighted linear sums [WA2|WK2], [WA|WK]
    W2 = g.add(g.cmul(M4, A5K5), g.cmul(M5, A6K6))
    W = g.add(g.cmul(M23, EK3), W2)
    WK2 = g.half(W2, 1)
    WA2 = g.half(W2, 0)

    # M row 0 (joint 1 uses (B, A) plane)
    bk6 = g.mul(B6, K6)
    bk2 = g.mul(B6, K2)
    k63 = g.sub(K6, K3)
    M01 = g.add(g.add(g.cmul(-M3 * D4, K3), g.cmul(-M4 * D4, K5)),
                g.cmul(-M5, bk6))
    M02 = g.add(g.add(M01, g.cmul((M3 + M4) * D4, K2)), g.cmul(M5, bk2))
    M03 = g.add(g.cmul(M4 * D4 * D5, c234), g.cmul(-M5, g.mul(B6, k63)))
    as5 = g.mul(A6, s5)
    bc45 = g.mul(B6, c45)
    M04 = g.add(g.cmul(M5, as5), g.cmul(-M5, bc45))

    # M block j,k in {1,2,3}
    Q2 = g.fold(sq2, "add")
    tt12 = g.mul(A2K2, W)                          # [t2|t1]
    u12 = g.fold(tt12, "add")
    M12 = g.sub(g.sub(M11nr, g.cmul(M1, Q2)), u12)
    M22 = g.add(g.add(M11, g.cmul(M23 + M45 - M1, Q2)), g.cmul(-2.0, u12))
    tt34 = g.mul(EK3, W2)                          # [t4|t3]
    u34 = g.fold(tt34, "add")
    M13 = g.sub(S45, u34)
    Q3 = g.fold(sq3, "add")
    M33 = g.add(g.add(S45, g.cmul(-2.0, u34)),
                g.ts2(Q3, M45, "mult", ROTOR, "add"))
    tt56 = g.mul(A2K2, W2)                         # [t6|t5]
    u56 = g.fold(tt56, "add")
    aekk = g.mul(A2K2, EK3)                        # [ae|kk]
    v = g.fold(aekk, "add")
    M23e = g.add(g.sub(M13, u56), g.cmul(M45, v))

    # M column 4 (joint 5); M44 is a constant
    P1 = g.fold(g.mul(g.rev(A6K6), cs234), "sub")  # K6*c234 - A6*s234
    P2 = g.fold(g.mul(g.rev(A2K2), cs234), "sub")
    P3 = g.fold(g.mul(g.rev(EK3), cs234), "sub")
    M14 = g.cmul(M5, g.mul(c5, P1))
    M24 = g.sub(M14, g.cmul(M5, g.mul(c5, P2)))
    M34 = g.sub(M14, g.cmul(M5, g.mul(c5, P3)))
    M44C = M5 + ROTOR / (D6 * D6)

    # direction to hand in the rotated frame; [dx|dz] pair
    hxr = g.add(g.mul(c1, hx), g.mul(s1, hy))
    hyr = g.sub(g.mul(s1, hx), g.mul(c1, hy))
    hzr = g.add(hz, g.C(-D1))
    hp = g.pk(hxr, hzr)
    dxdz = g.sub(hp, A6K6)                         # [dx|dz] wide sub
    dx = g.half(dxdz, 0)
    dz = g.half(dxdz, 1)
    dy = g.sub(hyr, B6)
    sqd = g.sq(dxdz)
    n2 = g.add(g.fold(sqd, "add"), g.sq(dy))

    # vd = Je^T d
    vd0 = g.sub(g.mul(A6, dy), g.mul(B6, dx))
    vd1 = g.fold(g.mul(g.rev(A6K6), dxdz), "sub")  # K6*dx - A6*dz
    d62 = g.rev(g.sub(A6K6, A2K2))                 # [K6-K2 | A6-A2]
    vd2 = g.fold(g.mul(d62, dxdz), "sub")
    d63 = g.rev(g.sub(A6K6, EK3))
    vd3 = g.fold(g.mul(d63, dxdz), "sub")
    cd45 = g.mul(c45s45, dxdz)                     # [c45*dx | s45*dz]
    # joint-5 coordinate rescaled by 1/d6: s is invariant when vd4, M[:,4]
    # and M44 are scaled consistently, so the d6 factors fold into constants
    vd4 = g.add(g.fold(cd45, "add"), g.mul(s5, dy))
    vd = [vd0, vd1, vd2, vd3, vd4]

    M = {(0, 0): M00, (0, 1): M01, (0, 2): M02, (0, 3): M03, (0, 4): M04,
         (1, 1): M11, (1, 2): M12, (1, 3): M13, (1, 4): M14,
         (2, 2): M22, (2, 3): M23e, (2, 4): M24,
         (3, 3): M33, (3, 4): M34}

    # Bordered LDL^T on [[M, vd], [vd^T, 0]] (6x6).  No sqrt: the pivot
    # chain (d -> recip -> C/L updates -> d) stays entirely on DVE.  The
    # last pivot d5 = -vd^T M^{-1} vd = -s, so the solve is integrated.
    Mb = dict(M)
    for j in range(5):
        Mb[(j, 5)] = vd[j]
    C = {}   # C[k,j] = L[k,j] * d_j (unnormalized column entries)
    L = {}   # normalized
    r = []
    for jc in range(5):
        if jc == 0:
            dd = Mb[(0, 0)]
        elif jc == 4:
            # M[4,4] is the constant M44C
            dd = g.ts2(g.mul(C[(4, 0)], L[(4, 0)]), -1.0, "mult", M44C, "add")
            for t in range(1, 4):
                dd = g.sub(dd, g.mul(C[(4, t)], L[(4, t)]))
        else:
            dd = Mb[(jc, jc)]
            for t in range(jc):
                dd = g.sub(dd, g.mul(C[(jc, t)], L[(jc, t)]))
        rj = g.recip(dd)
        r.append(rj)
        for kk2 in range(jc + 1, 6):
            a = Mb[(jc, kk2)]
            for t in range(jc):
                a = g.sub(a, g.mul(C[(kk2, t)], L[(jc, t)]))
            C[(kk2, jc)] = a
            L[(kk2, jc)] = g.mul(a, rj)
    # s = sum_t C[5,t]*L[5,t]
    sacc = None
    for t in range(5):
        p = g.mul(C[(5, t)], L[(5, t)])
        sacc = p if sacc is None else g.add(sacc, p)
    cost_neg = g.mul(g.cmul(-1.0, g.recip(sacc)), n2)
    g.finalize_widths()
    return g, cost_neg


# ----------------------------------------------------------------------------
# numpy evaluation of the DAG (for validation in test.py)
# ----------------------------------------------------------------------------

def eval_numpy(g, root, chans):
    """Width-2 node values are tuples (left, right) of arrays."""
    val = {}

    def f32c(v):
        if isinstance(v, tuple):
            return tuple(x.astype(np.float32) for x in v)
        return v.astype(np.float32)

    for n in g.nodes:
        if n.op == "const":
            val[n.id] = np.float32(n.c)
            continue
        if n.op == "in":
            val[n.id] = chans[n.c].astype(np.float32)
            continue
        a = [val[x.id] for x in n.args]
        if n.width >= 2 and n.op not in ("pack", "rev", "half", "vslice",
                                         "fold"):
            w = n.width
            a = [(x,) * w if not isinstance(x, tuple) else x for x in a]

            def bop(f):
                return tuple(f(a[0][i], a[1][i]) for i in range(w))

            def uop(f):
                return tuple(f(a[0][i]) for i in range(w))
        else:
            def bop(f):
                return f(a[0], a[1])

            def uop(f):
                return f(a[0])

        if n.op == "pack":
            v = tuple(a)
        elif n.op == "rev":
            v = (a[0][1], a[0][0])
        elif n.op == "half":
            v = a[0][n.c]
        elif n.op == "vslice":
            v = a[0][n.c[0]:n.c[0] + n.c[1]]
        elif n.op == "fold":
            op, swap = n.c
            l, r = a[0]
            if swap:
                l, r = r, l
            v = (l + r) if op == "add" else (l - r)
        elif n.op == "add":
            v = bop(lambda x, y: x + y)
        elif n.op == "sub":
            v = bop(lambda x, y: x - y)
        elif n.op == "mul":
            v = bop(lambda x, y: x * y)
        elif n.op == "square":
            v = uop(lambda x: x * x)
        elif n.op == "cmul":
            v = uop(lambda x: np.float32(n.c) * x)
        elif n.op == "cadd":
            v = uop(lambda x: x + np.float32(n.c))
        elif n.op == "sin":
            sc, b = n.c
            v = uop(lambda x: np.sin(np.float32(sc) * x + np.float32(b)))
        elif n.op == "ts2":
            s1, op0, s2, op1 = n.c

            def ts2f(x):
                for s_, o_ in ((s1, op0), (s2, op1)):
                    if o_ == "mult":
                        x = x * np.float32(s_)
                    elif o_ == "is_ge":
                        x = (x >= np.float32(s_)).astype(np.float32)
                    else:
                        x = x + np.float32(s_)
                return x
            v = uop(ts2f)
        elif n.op == "sqrt":
            v = uop(np.sqrt)
        elif n.op == "recip":
            v = uop(lambda x: np.float32(1.0) / x)
        else:
            raise ValueError(n.op)
        val[n.id] = f32c(v)
    return val[root.id]


def ref_numpy(x):
    """Full-pipeline numpy reference using the DAG; x [B,H,26] -> [B]."""
    B, H, Cc = x.shape
    N = B * H
    flat = x.reshape(N, Cc).astype(np.float32)
    g, root = build_graph()
    chans = {i: flat[:, SRC_COLS[i]] for i in range(8)}
    cn = eval_numpy(g, root, chans)
    return cn.reshape(B, H).sum(axis=1)


# ----------------------------------------------------------------------------
# planning: STT fusion + ETF list scheduling across dve/act/gps
# ----------------------------------------------------------------------------

# pipelined per-[128,128]-op costs (TimelineSim probe)
COST = {
    ("dve", "tt"): 212.0, ("dve", "stt"): 212.0, ("dve", "ts"): 162.0,
    ("dve", "recip"): 204.0, ("dve", "reduce"): 296.0,
    ("act", "any"): 360.0,
    ("gps", "tt"): 440.0, ("gps", "ts"): 360.0,
}
XLAT = 100.0  # cross-engine semaphore latency


# per-width op costs: measured w=1 base + per-extra-slot slope
def wcost(base, slope, w):
    return base + slope * (w - 1)


def classify(n):
    """Returns options = [(engine, cost), ...]. GPSIMD (Pool) supports only
    tensor_tensor and tensor_scalar; scalar_tensor_tensor is DVE-only."""
    w = n.width
    c_tt = wcost(212.0, 144.0, w)
    c_ts = wcost(162.0, 89.0, w)
    c_act = wcost(360.0, 106.0, w)
    c_gtt = wcost(440.0, 260.0, w)
    c_gts = wcost(360.0, 151.0, w)
    if n.op == "sin" or n.op == "sqrt":
        return [("act", c_act)]
    if n.op == "recip":
        return [("dve", COST[("dve", "recip")])]
    if n.op == "fold":
        return [("dve", COST[("dve", "tt")]), ("gps", COST[("gps", "tt")])]
    if n.op == "square":
        return [("dve", c_tt), ("act", c_act), ("gps", c_gtt)]
    if n.op in ("cadd", "cmul", "ts2"):
        # ts2 with non-(mult,add) pattern can't be an ACT Copy
        actok = True
        if n.op == "ts2" and (n.c[1], n.c[3]) != ("mult", "add"):
            actok = False
        opts = [("dve", c_ts)]
        if actok:
            opts.append(("act", c_act))
        opts.append(("gps", c_gts))
        return opts
    if n.op in ("add", "sub", "mul"):
        if isinstance(n.c, tuple) and n.c and n.c[0] == "stt_cmul":
            return [("dve", c_tt)]
        return [("dve", c_tt), ("gps", c_gtt)]
    if n.op == "cmul_stt":  # cmul fused with mul/square arg
        return [("dve", c_tt)]
    raise ValueError(n.op)


def plan(g, root):
    """STT fusion + ETF scheduling. Returns emit list ordered by virtual
    start time, with n.engine set."""
    # reachability + users
    reach = set()
    stack = [root]
    while stack:
        n = stack.pop()
        if n.id in reach:
            continue
        reach.add(n.id)
        stack.extend(n.args)
    for n in g.nodes:
        n.users = []
    order = [n for n in g.nodes if n.id in reach]
    for n in order:
        for a in n.args:
            a.users.append(n)

    VIEWS = ("pack", "rev", "half", "vslice")

    # fusion: add/sub(x, cmul(c,y)) -> STT ; cmul(c, mul(x,y)/square(x)) -> STT
    # (never fuse away a pack member: its output must land in the pack tile)
    for n in order:
        if n.op in VIEWS:
            continue
        if n.op in ("add", "sub"):
            for k, a in enumerate(n.args):
                if a.op == "cmul" and len(a.users) == 1 and a.fused_into is None \
                        and a.pack_into is None \
                        and a.args[0].fused_into is None \
                        and a.args[0].op != "const":
                    n.c = ("stt_cmul", k, a.c)
                    a.fused_into = n
                    break
        elif n.op == "cmul" and n.fused_into is None:
            a = n.args[0]
            if a.op in ("mul", "square") and len(a.users) == 1 \
                    and a.fused_into is None and a.pack_into is None \
                    and all(aa.fused_into is None for aa in a.args):
                a.fused_into = n

    # effective deps of an emitted node (through fused producers and views)
    def resolve(a, out):
        if a.op in ("const", "in"):
            return
        if a.op == "pack":
            for m in a.args:
                resolve(m, out)
        elif a.op in ("rev", "half", "vslice"):
            resolve(a.args[0], out)
        else:
            out.append(a)

    def deps(n):
        out = []
        for a in n.args:
            if a.fused_into is n:
                for aa in a.args:
                    resolve(aa, out)
            else:
                resolve(a, out)
        return out

    emit_nodes = [n for n in order
                  if n.op not in ("const", "in") and n.op not in VIEWS
                  and n.fused_into is None]

    # ts-class ops occurring after the trig preamble go to the otherwise
    # idle ACT engine (its eligible work is inherently front-loaded)
    max_sin = max((n.id for n in emit_nodes if n.op == "sin"), default=0)

    def opts_of(n):
        if n.op == "cmul" and n.args[0].fused_into is n:
            e = Expr("cmul_stt")
            e.width = n.width
            return classify(e)
        return classify(n)

    # critical-path priority (min cost per node)
    mincost = {n.id: min(c for _, c in opts_of(n)) for n in emit_nodes}
    prio = {}

    def get_prio(n):
        if n.id in prio:
            return prio[n.id]
        p = mincost[n.id] + max(
            (get_prio(u if u.fused_into is None else u.fused_into)
             for u in n.users if (u.fused_into is None or u.fused_into is not n)
             ), default=0.0)
        prio[n.id] = p
        return p

    for n in order:
        n.prio = 0.0
    # prios in reverse topological order; views are zero-cost pass-throughs
    for n in reversed(order):
        if n.op in ("const", "in"):
            continue
        best = 0.0
        for u in n.users:
            tgt = u.fused_into if u.fused_into is not None else u
            if tgt is n:
                continue
            if tgt.op not in ("const", "in"):
                best = max(best, tgt.prio)
        own = mincost[n.id] if n.id in mincost else 0.0
        n.prio = own + best

    # ---- phase 1: static engine assignment (balance max load) ----
    # Critical-chain nodes keep their fastest engine; the rest greedily go to
    # the engine with the smallest resulting load.
    ndeps = {n.id: 0 for n in emit_nodes}
    dep_lists = {}
    for n in emit_nodes:
        dl = deps(n)
        dep_lists[n.id] = dl
        ndeps[n.id] = len(dl)
    users_emit = {n.id: [] for n in emit_nodes}
    for n in emit_nodes:
        for d in dep_lists[n.id]:
            users_emit[d.id].append(n)

    def run_etf(gamma, win, xlat, act_disc=1.0):
        """ETF with load-penalty engine choice. Returns (makespan, sched:
        list of (n, engine, start, finish))."""
        nd = dict(ndeps)
        ready = [n for n in emit_nodes if nd[n.id] == 0]
        eng_free = {"dve": 0.0, "act": 0.0, "gps": 0.0}
        eload = {"dve": 0.0, "act": 0.0, "gps": 0.0}
        fin = {}
        eng_of = {}
        sched = []
        while ready:
            cands = []
            for n in ready:
                dr_cache = {}
                for e, c in opts_of(n):
                    dr = 0.0
                    for d in dep_lists[n.id]:
                        dr = max(dr, fin[d.id] +
                                 (xlat if eng_of[d.id] != e else 0.0))
                    st = max(eng_free[e], dr)
                    ceff = c * act_disc if e == "act" else c
                    score = st + ceff + gamma * eload[e]
                    cands.append((score, n.prio, n, e, c, st))
            smin = min(c[0] for c in cands)
            _, _, n, e, c, st = max(
                (cd for cd in cands if cd[0] <= smin + win),
                key=lambda cd: (cd[1], -cd[0]))
            ready.remove(n)
            fin[n.id] = st + c
            eng_of[n.id] = e
            eng_free[e] = st + c
            eload[e] += c
            sched.append((n, e, st, st + c))
            for u in users_emit[n.id]:
                nd[u.id] -= 1
                if nd[u.id] == 0:
                    ready.append(u)
        return max(f for _, _, _, f in sched), sched

    best_ms, best_sched = None, None
    for gamma in (0.0, 0.02, 0.05, 0.1, 0.2, 0.4, 0.7):
        for win in (0.0, 80.0, 150.0, 250.0):
            for xl in (200.0,):
                for ad in (0.6,):
                    ms, sched = run_etf(gamma, win, xl, ad)
                    if best_ms is None or ms < best_ms:
                        best_ms, best_sched = ms, sched

    load = {"dve": 0.0, "act": 0.0, "gps": 0.0}
    for n, e, st, f in best_sched:
        n.engine = e
        n.start = st
        n.finish = f
        load[e] += f - st

    scheduled = [n for n, _, _, _ in best_sched]
    scheduled.sort(key=lambda n: (n.start, n.finish))
    for i2, n in enumerate(scheduled):
        n.order = i2
    makespan = best_ms
    return scheduled, load, makespan


# ----------------------------------------------------------------------------
# bass emission
# ----------------------------------------------------------------------------

NCORES = 8
B_FULL, H, CH = 2048, 64, 26
N_PER_CORE = B_FULL * H // NCORES          # 16384
P = 128
FD = N_PER_CORE // P                        # 128
NCH = 8


def _build_bass():
    import concourse.bass as bass
    from concourse.bacc import Bacc
    import concourse.mybir as mybir
    from concourse.tile import TileContext

    f32 = mybir.dt.float32
    alu = mybir.AluOpType
    AF = mybir.ActivationFunctionType

    g, root = build_graph()
    emit, load, makespan = plan(g, root)

    nc = Bacc()
    xs = nc.dram_tensor("xs", (P, NCH * FD), f32, kind="ExternalInput")
    out = nc.dram_tensor("out", (B_FULL // NCORES,), f32, kind="ExternalOutput")

    # liveness for slot allocation
    last_use = {}
    for n in emit:
        for a in n.args:
            if a.order is not None:
                last_use[a.id] = max(last_use.get(a.id, -1), n.order)
            if a.fused_into is n:
                for aa in a.args:
                    if aa.order is not None:
                        last_use[aa.id] = max(last_use.get(aa.id, -1), n.order)
    last_use[root.id] = len(emit) + 10

    with TileContext(nc) as tc:
        with tc.tile_pool(name="vals", bufs=1) as vp:
            # three staged input groups: [q2 q3 q4] [q1 q5] [hx hy hz],
            # issued at t=0 on three different HWDGE-capable engines so the
            # fixed DGE latencies overlap; transfers serialize on the DMA bus
            # in issue order (q2/q3/q4 first — head of the trig chain).
            stA = vp.tile([P, 3 * FD], f32, tag="stA", name="stA")
            stB = vp.tile([P, 2 * FD], f32, tag="stB", name="stB")
            stC = vp.tile([P, 3 * FD], f32, tag="stC", name="stC")
            nc.gpsimd.dma_start(stA[:, :], xs[:, 0:3 * FD])
            nc.sync.dma_start(stB[:, :], xs[:, 3 * FD:5 * FD])
            nc.scalar.dma_start(stC[:, :], xs[:, 5 * FD:8 * FD])
            # const APs for non-Copy activation biases (registered after the
            # DMAs so they don't delay them; barrier orders memset vs readers)
            for cv in (PI / 2,):
                t = nc.alloc_sbuf_tensor(f"constf32-{cv}", [128, 1], f32)
                nc.gpsimd.memset(t.ap(), cv)
                nc.const_aps.aps[(f32, float(cv))] = t.ap()
            nc.all_engine_barrier()

            def chan_ap(ch):
                if ch < 3:
                    return stA[:, ch * FD:(ch + 1) * FD]
                if ch < 5:
                    return stB[:, (ch - 3) * FD:(ch - 2) * FD]
                return stC[:, (ch - 5) * FD:(ch - 4) * FD]

            from collections import deque
            free_slots = deque()
            SLACK = 60
            n_slots = [0]
            w_slots = [0]
            node_tile = {}   # id -> (tile, col_lo, ncols)
            pack_tile = {}

            def desc_of(n):
                """(tile, col_lo, ncols) for a value-holding node."""
                if n.op == "pack":
                    if n.id not in pack_tile:
                        t = vp.tile([P, n.width * FD], f32, tag=f"pk{n.id}",
                                    name=f"pk{n.id}", bufs=1)
                        pack_tile[n.id] = t
                    return (pack_tile[n.id], 0, n.width * FD)
                if n.op == "half":
                    t, lo, w = desc_of(n.args[0])
                    return (t, lo + n.c * FD, FD)
                if n.op == "vslice":
                    t, lo, w = desc_of(n.args[0])
                    return (t, lo + n.c[0] * FD, n.c[1] * FD)
                if n.op == "rev":
                    return desc_of(n.args[0])
                return node_tile[n.id]

            def ap2(n):
                if n.op == "in":
                    return chan_ap(n.c)
                t, lo, w = desc_of(n)
                return t[:, lo:lo + w]

            def apw(n, w):
                """[P, w, FD] view: wide node, reversed pair, or broadcast."""
                if n.op == "rev":
                    return apw(n.args[0], 2)[:, ::-1, :]
                if n.width == w:
                    return ap2(n).rearrange("p (c q) -> p c q", c=w)
                assert n.width == 1
                return ap2(n).unsqueeze(1).broadcast_to([P, w, FD])

            def alloc(n):
                if n.pack_into is not None:
                    pk, slot = n.pack_into
                    t, lo, w = desc_of(pk)
                    node_tile[n.id] = (t, lo + slot * FD, FD)
                    return t[:, slot * FD:(slot + 1) * FD]
                if n.width > 1:
                    sl = w_slots[0]
                    w_slots[0] += 1
                    t = vp.tile([P, n.width * FD], f32, tag=f"w{sl}x{n.width}",
                                name=f"v{n.id}", bufs=1)
                    node_tile[n.id] = (t, 0, n.width * FD)
                    return t[:, :]
                if len(free_slots) > SLACK:
                    sl = free_slots.popleft()
                else:
                    sl = n_slots[0]
                    n_slots[0] += 1
                t = vp.tile([P, FD], f32, tag=f"s{sl}", name=f"v{n.id}", bufs=2)
                n.slot = sl
                node_tile[n.id] = (t, 0, FD)
                return t[:, :]

            by_last = {}
            for nid, lu in last_use.items():
                by_last.setdefault(lu, []).append(nid)

            eng = {"dve": nc.vector, "act": nc.scalar, "gps": nc.gpsimd}
            ALU_OF = {"add": alu.add, "sub": alu.subtract, "mul": alu.mult}

            def needs3(n, tens_args):
                if n.width < 2:
                    return False
                return any(a.op == "rev" or a.width != n.width
                           for a in tens_args if a.op not in ("const",))

            def aps_for(n, ot2, tens_args):
                """Return (out_ap, [arg_aps]) with matching dimensionality."""
                if needs3(n, tens_args):
                    o3 = ot2.rearrange("p (c q) -> p c q", c=n.width)
                    return o3, [apw(a, n.width) for a in tens_args]
                return ot2, [ap2(a) for a in tens_args]

            for n in emit:
                ot = alloc(n)
                e = eng[n.engine]
                en = n.engine
                if n.op == "sin":
                    sc, b = n.c
                    nc.scalar.activation(ot, ap2(n.args[0]), AF.Sin,
                                         bias=float(b), scale=float(sc))
                elif n.op == "sqrt":
                    nc.scalar.activation(ot, ap2(n.args[0]), AF.Sqrt)
                elif n.op == "recip":
                    nc.vector.reciprocal_approx_fast(out=ot, in_=ap2(n.args[0]))
                elif n.op == "fold":
                    fop, swap = n.c
                    t, lo, w = desc_of(n.args[0])
                    l = t[:, lo:lo + FD]
                    r = t[:, lo + FD:lo + 2 * FD]
                    if swap:
                        l, r = r, l
                    e.tensor_tensor(ot, l, r,
                                    alu.add if fop == "add" else alu.subtract)
                elif n.op == "square":
                    oa, (ia,) = aps_for(n, ot, [n.args[0]])
                    if en == "act":
                        nc.scalar.activation(oa, ia, AF.Square)
                    else:
                        e.tensor_tensor(oa, ia, ia, alu.mult)
                elif n.op == "cadd":
                    oa, (ia,) = aps_for(n, ot, [n.args[0]])
                    if en == "act":
                        nc.scalar.activation(oa, ia, AF.Copy,
                                             bias=float(n.c), scale=1.0)
                    else:
                        e.tensor_scalar_add(oa, ia, float(n.c))
                elif n.op == "ts2":
                    s1, op0, s2, op1 = n.c
                    oa, (ia,) = aps_for(n, ot, [n.args[0]])
                    if en == "act":
                        nc.scalar.activation(oa, ia, AF.Copy,
                                             bias=float(s2), scale=float(s1))
                    else:
                        e.tensor_scalar(oa, ia, float(s1), float(s2),
                                        getattr(alu, op0), getattr(alu, op1))
                elif n.op == "cmul":
                    a = n.args[0]
                    if a.fused_into is n:
                        if a.op == "square":
                            x = yv = a.args[0]
                        else:
                            x, yv = a.args
                        oa, (xa, ya) = aps_for(n, ot, [x, yv])
                        e.scalar_tensor_tensor(oa, xa, float(n.c),
                                               ya, alu.mult, alu.mult)
                    else:
                        oa, (ia,) = aps_for(n, ot, [a])
                        if en == "act":
                            nc.scalar.activation(oa, ia, AF.Copy,
                                                 bias=0.0, scale=float(n.c))
                        else:
                            e.tensor_scalar_mul(oa, ia, float(n.c))
                elif n.op in ("add", "sub"):
                    if isinstance(n.c, tuple) and n.c and n.c[0] == "stt_cmul":
                        _, k, cval = n.c
                        cm = n.args[k]
                        other = n.args[1 - k]
                        x = cm.args[0]
                        oa, (xa, ya) = aps_for(n, ot, [x, other])
                        if n.op == "add":
                            e.scalar_tensor_tensor(oa, xa, float(cval),
                                                   ya, alu.mult, alu.add)
                        elif k == 1:
                            e.scalar_tensor_tensor(oa, xa, float(-cval),
                                                   ya, alu.mult, alu.add)
                        else:
                            e.scalar_tensor_tensor(oa, xa, float(cval),
                                                   ya, alu.mult, alu.subtract)
                    else:
                        oa, (xa, ya) = aps_for(n, ot, [n.args[0], n.args[1]])
                        e.tensor_tensor(oa, xa, ya, ALU_OF[n.op])
                elif n.op == "mul":
                    oa, (xa, ya) = aps_for(n, ot, [n.args[0], n.args[1]])
                    e.tensor_tensor(oa, xa, ya, alu.mult)
                else:
                    raise ValueError(n.op)

                for nid in by_last.get(n.order, []):
                    nd = g.nodes[nid]
                    if nd.slot is not None and nd.id != root.id:
                        free_slots.append(nd.slot)
                        nd.slot = None

            # epilogue: per-b sums (64-sample segments)
            osum = vp.tile([P, 2], f32, tag="osum", bufs=2)
            ct, clo, _cw = node_tile[root.id]
            cr3 = ct[:, clo:clo + 128].rearrange("p (j q) -> p j q", j=2)
            nc.vector.tensor_reduce(osum[:, 0:2].unsqueeze(2), cr3,
                                    mybir.AxisListType.X, alu.add)
            nc.gpsimd.dma_start(out.rearrange("(p j) -> p j", p=P), osum[:, :])

    nc.compile()
    return nc, len(emit), load, makespan


_CACHE = {}


def kernel(x, cond, time):
    from concourse.bass_utils import run_bass_kernel_spmd

    if "nc" not in _CACHE:
        nc, n_ops, load, makespan = _build_bass()
        _CACHE["nc"] = nc
        _CACHE["stats"] = (n_ops, load, makespan)
    nc = _CACHE["nc"]

    xf = np.ascontiguousarray(x, dtype=np.float32).reshape(B_FULL * H, CH)
    sel = xf[:, SRC_COLS]                                   # [131072, 8]
    in_maps = []
    for k in range(NCORES):
        shard = sel[k * N_PER_CORE:(k + 1) * N_PER_CORE]    # [16384, 8]
        # [128 p, 128 q, 8 c] -> [128, 8, 128] channel-major free layout
        arr = shard.reshape(P, FD, NCH).transpose(0, 2, 1)
        in_maps.append({"xs": np.ascontiguousarray(arr).reshape(P, NCH * FD)})
    res = run_bass_kernel_spmd(nc, in_maps, core_ids=list(range(NCORES)))
    _CACHE["exec_time_ns"] = res.exec_time_ns
    _CACHE["trace"] = res.instructions_and_trace
    outs = [res.results[k]["out"] for k in range(NCORES)]
    return np.concatenate(outs).astype(np.float32)


if __name__ == "__main__":
    g, root = build_graph()
    emit, load, makespan = plan(g, root)
    from collections import Counter
    print("emitted ops:", len(emit))
    print(Counter((n.engine, n.op) for n in emit))
    print("load est (us):", {k: v / 1000 for k, v in load.items()})
    print("virtual makespan (us):", makespan / 1000)


# revision 71
# speedup vs baseline: 1.0120x; 1.0120x over previous
"""Trainium2 Bass kernel for the UR5e reflected-mass cost function.

Closed-form math (per sample n of 131072 = 2048 b x 64 h):
  The last joint (q6) never affects the output (its Jacobian column is 0),
  and in the q1-rotated "cylindrical" frame every frame origin is
  p_i = (A_i, B_i, C_i) with the z-axes {z0=ez, z1=z2=z3=(0,1,0),
  z4=(s234,0,-c234)}.  All Jacobian columns, the 5x5 mass matrix, and the
  end-effector direction reduce to ~260 scalar ops instead of the naive
  ~670 of the frame-by-frame DH chain.

Implementation: every per-sample scalar is a [128,128] f32 SBUF tile
(16384 samples per core, 8 cores data-parallel over b).  The computation
is a symbolic scalar DAG with CSE + constant folding + STT fusion,
scheduled onto the DVE/ACT/GPSIMD engines with an earliest-finish-time
list scheduler and emitted through the Tile framework.
"""

import math
import numpy as np

# ----------------------------------------------------------------------------
# constants
# ----------------------------------------------------------------------------

PI = math.pi
A2C, A3C = -0.425, -0.3922
D1, D4, D5, D6 = 0.1625, 0.1333, 0.0997, 0.0996
# LINK_MASS[i] sits at frame origin p_{i+1}; link 0 (at p1) never moves.
M1, M2, M3, M4, M5 = 8.058, 2.846, 1.37, 1.3, 0.365
M23 = M2 + M3
M45 = M4 + M5
ROTOR = 0.1
MAGIC = 12582912.0  # 1.5 * 2**23 f32 round-to-int trick

# host channel order handed to the device
# 0:q2 1:q3 2:q4 3:q1 4:q5 5:hx 6:hy 7:hz
SRC_COLS = [7, 8, 9, 6, 10, 19, 20, 21]

# ----------------------------------------------------------------------------
# symbolic scalar DAG
# ----------------------------------------------------------------------------


class Expr:
    __slots__ = ("op", "args", "c", "id", "users", "engine", "fused_into",
                 "slot", "order", "prio", "start", "finish", "width",
                 "pack_into")

    def __init__(self, op, args=(), c=None, i=0):
        self.op = op
        self.args = args
        self.c = c
        self.id = i
        self.users = []
        self.engine = None
        self.fused_into = None
        self.slot = None
        self.order = None
        self.prio = 0.0
        self.start = 0.0
        self.finish = 0.0
        self.width = 1
        self.pack_into = None  # (pack_node, slot) for co-located members


class Graph:
    def __init__(self):
        self.nodes = []
        self.cse = {}

    def _mk(self, op, args=(), c=None):
        key = (op, tuple(a.id for a in args), c)
        n = self.cse.get(key)
        if n is None:
            n = Expr(op, args, c, len(self.nodes))
            self.nodes.append(n)
            self.cse[key] = n
        return n

    def C(self, v):
        return self._mk("const", c=float(v))

    def IN(self, ch):
        return self._mk("in", c=ch)

    def add(self, x, y):
        if x.op == "const" and y.op == "const":
            return self.C(x.c + y.c)
        if x.op == "const":
            x, y = y, x
        if y.op == "const":
            if y.c == 0.0:
                return x
            return self._mk("cadd", (x,), y.c)
        a, b = (x, y) if x.id <= y.id else (y, x)
        return self._mk("add", (a, b))

    def sub(self, x, y):
        if x.op == "const" and y.op == "const":
            return self.C(x.c - y.c)
        if y.op == "const":
            if y.c == 0.0:
                return x
            return self._mk("cadd", (x,), -y.c)
        if x.op == "const" and x.c == 0.0:
            return self.cmul(-1.0, y)
        if x is y:
            return self.C(0.0)
        return self._mk("sub", (x, y))

    def cmul(self, c, x):
        c = float(c)
        if x.op == "const":
            return self.C(c * x.c)
        if c == 0.0:
            return self.C(0.0)
        if c == 1.0:
            return x
        if x.op == "cmul":
            return self.cmul(c * x.c, x.args[0])
        return self._mk("cmul", (x,), c)

    def mul(self, x, y):
        if x.op == "const":
            return self.cmul(x.c, y)
        if y.op == "const":
            return self.cmul(y.c, x)
        if x.op == "cmul" and y.op == "cmul":
            return self.cmul(x.c * y.c, self.mul(x.args[0], y.args[0]))
        if x.op == "cmul":
            return self.cmul(x.c, self.mul(x.args[0], y))
        if y.op == "cmul":
            return self.cmul(y.c, self.mul(x, y.args[0]))
        if x is y:
            return self._mk("square", (x,))
        a, b = (x, y) if x.id <= y.id else (y, x)
        return self._mk("mul", (a, b))

    def ts2(self, x, s1, op0, s2, op1):
        return self._mk("ts2", (x,), (float(s1), op0, float(s2), op1))

    def sincos(self, q):
        """(sin q, cos q) sharing one range reduction.
        r0 = q - 2*pi*round(q/2pi) in [-pi, pi]; sin = Sin(r0).
        cos = Sin(r0c + pi/2) where r0c = r0 - 2pi*(r0 >= pi/2), keeping the
        Sin argument in [-pi, pi]."""
        inv2pi = 1.0 / (2.0 * PI)
        t1 = self.ts2(q, inv2pi, "mult", MAGIC, "add")
        k = self._mk("cadd", (t1,), -MAGIC)
        r0 = self.add(self.cmul(-2.0 * PI, k), q)  # fuses to one STT
        s = self._mk("sin", (r0,), (1.0, 0.0))
        ge = self._mk("ts2", (r0,), (PI / 2, "is_ge", 1.0, "mult"))
        r0c = self.add(self.cmul(-2.0 * PI, ge), r0)  # STT
        c = self._mk("sin", (r0c,), (1.0, PI / 2))
        return s, c

    def sqrt_(self, x):
        return self._mk("sqrt", (x,))

    def recip(self, x):
        return self._mk("recip", (x,))

    def sq(self, x):
        return self._mk("square", (x,))

    # ---- wide (width-n) machinery ----
    def pk(self, *members):
        """Co-locate width-1 emitted ops into one [128, n*128] tile.
        Free: members write directly into the pack's tile slots."""
        for m in members:
            assert m.op not in ("const", "in")
        p = self._mk("pack", tuple(members))
        p.width = len(members)
        for i, m in enumerate(members):
            m.pack_into = (p, i)
        return p

    def rev(self, p):
        """Swapped-halves view of a pair (negative-stride AP). Free."""
        n = self._mk("rev", (p,))
        n.width = 2
        return n

    def half(self, p, i):
        """View of one slot of a wide node. Free."""
        n = self._mk("half", (p,), i)
        return n

    def vslice(self, p, lo, w):
        """View of w contiguous slots [lo, lo+w) of a wide node. Free."""
        n = self._mk("vslice", (p,), (lo, w))
        n.width = w
        return n

    def fold(self, p, op, swap=False):
        """[128,128] result = left op right (or right op left) of a pair."""
        return self._mk("fold", (p,), (op, bool(swap)))

    def finalize_widths(self):
        for n in self.nodes:
            if n.op == "pack":
                n.width = len(n.args)
            elif n.op == "rev":
                n.width = 2
            elif n.op == "vslice":
                n.width = n.c[1]
            elif n.op in ("half", "fold"):
                n.width = 1
            elif n.args:
                n.width = max([a.width for a in n.args] + [1])


def build_graph():
    """Returns (graph, cost_neg_node). cost_neg = -cost per sample."""
    g = Graph()
    q2, q3, q4, q1, q5 = (g.IN(i) for i in range(5))
    hx, hy, hz = (g.IN(5 + i) for i in range(3))

    q23 = g.add(q2, q3)
    q234 = g.add(q23, q4)
    s1, c1 = g.sincos(q1)
    s2, c2 = g.sincos(q2)
    s23, c23 = g.sincos(q23)
    s234, c234 = g.sincos(q234)
    s5, c5 = g.sincos(q5)

    # cylindrical coordinates in (A|K) pairs (A1 = K1 = 0, K = C - d1)
    cs2 = g.pk(c2, s2)
    cs23 = g.pk(c23, s23)
    cs234 = g.pk(c234, s234)
    A2K2 = g.cmul(A2C, cs2)                       # [A2|K2] wide ts
    EK3 = g.add(A2K2, g.cmul(A3C, cs23))          # [E|K3] wide STT
    A2 = g.half(A2K2, 0)
    K2 = g.half(A2K2, 1)
    E = g.half(EK3, 0)
    K3 = g.half(EK3, 1)
    ccsc = g.mul(cs234, s5)                       # [cc|sc] broadcast mul
    c45s45 = g.mul(cs234, c5)                     # [c45|s45] broadcast mul
    c45 = g.half(c45s45, 0)
    s45 = g.half(c45s45, 1)
    # A5 = E + d5*s234 ; K5 = K3 - d5*c234  (different signs: packed scalars)
    A5 = g.add(E, g.cmul(D5, s234))
    K5 = g.sub(K3, g.cmul(D5, c234))
    A5K5 = g.pk(A5, K5)
    A6K6 = g.sub(A5K5, g.cmul(D6, ccsc))          # wide STT
    A6 = g.half(A6K6, 0)
    K6 = g.half(A6K6, 1)
    B6 = g.ts2(c5, D6, "mult", D4, "add")         # B6 = d4 + d6*c5

    # squares (wide on ACT)
    sq2 = g.sq(A2K2)
    sq3 = g.sq(EK3)
    sq5 = g.sq(A5K5)
    sq6 = g.sq(A6K6)
    B6s = g.sq(B6)

    # weighted square sums [SA|SK] (suffix style so S45 comes free)
    SS45 = g.add(g.cmul(M5, sq6), g.cmul(M4, sq5))
    SS = g.add(g.add(SS45, g.cmul(M23, sq3)), g.cmul(M1, sq2))
    SA = g.half(SS, 0)
    M11nr = g.fold(SS, "add")
    M11 = g.add(M11nr, g.C(ROTOR))
    M00 = g.add(g.add(SA, g.cmul(M5, B6s)), g.C((M3 + M4) * D4 * D4 + ROTOR))
    S45 = g.fold(SS45, "add")

    # weighted linear sums [WA2|WK2], [WA|WK]
    W2 = g.add(g.cmul(M4, A5K5), g.cmul(M5, A6K6))
    W = g.add(g.cmul(M23, EK3), W2)
    WK2 = g.half(W2, 1)
    WA2 = g.half(W2, 0)

    # M row 0 (joint 1 uses (B, A) plane)
    bk6 = g.mul(B6, K6)
    bk2 = g.mul(B6, K2)
    k63 = g.sub(K6, K3)
    M01 = g.add(g.add(g.cmul(-M3 * D4, K3), g.cmul(-M4 * D4, K5)),
                g.cmul(-M5, bk6))
    M02 = g.add(g.add(M01, g.cmul((M3 + M4) * D4, K2)), g.cmul(M5, bk2))
    M03 = g.add(g.cmul(M4 * D4 * D5, c234), g.cmul(-M5, g.mul(B6, k63)))
    as5 = g.mul(A6, s5)
    bc45 = g.mul(B6, c45)
    M04 = g.add(g.cmul(M5, as5), g.cmul(-M5, bc45))

    # M block j,k in {1,2,3}
    Q2 = g.fold(sq2, "add")
    tt12 = g.mul(A2K2, W)                          # [t2|t1]
    u12 = g.fold(tt12, "add")
    M12 = g.sub(g.sub(M11nr, g.cmul(M1, Q2)), u12)
    M22 = g.add(g.add(M11, g.cmul(M23 + M45 - M1, Q2)), g.cmul(-2.0, u12))
    tt34 = g.mul(EK3, W2)                          # [t4|t3]
    u34 = g.fold(tt34, "add")
    M13 = g.sub(S45, u34)
    Q3 = g.fold(sq3, "add")
    M33 = g.add(g.add(S45, g.cmul(-2.0, u34)),
                g.ts2(Q3, M45, "mult", ROTOR, "add"))
    tt56 = g.mul(A2K2, W2)                         # [t6|t5]
    u56 = g.fold(tt56, "add")
    aekk = g.mul(A2K2, EK3)                        # [ae|kk]
    v = g.fold(aekk, "add")
    M23e = g.add(g.sub(M13, u56), g.cmul(M45, v))

    # M column 4 (joint 5); M44 is a constant
    P1 = g.fold(g.mul(g.rev(A6K6), cs234), "sub")  # K6*c234 - A6*s234
    P2 = g.fold(g.mul(g.rev(A2K2), cs234), "sub")
    P3 = g.fold(g.mul(g.rev(EK3), cs234), "sub")
    M14 = g.cmul(M5, g.mul(c5, P1))
    M24 = g.sub(M14, g.cmul(M5, g.mul(c5, P2)))
    M34 = g.sub(M14, g.cmul(M5, g.mul(c5, P3)))
    M44C = M5 + ROTOR / (D6 * D6)

    # direction to hand in the rotated frame; [dx|dz] pair
    hxr = g.add(g.mul(c1, hx), g.mul(s1, hy))
    hyr = g.sub(g.mul(s1, hx), g.mul(c1, hy))
    hzr = g.add(hz, g.C(-D1))
    hp = g.pk(hxr, hzr)
    dxdz = g.sub(hp, A6K6)                         # [dx|dz] wide sub
    dx = g.half(dxdz, 0)
    dz = g.half(dxdz, 1)
    dy = g.sub(hyr, B6)
    sqd = g.sq(dxdz)
    n2 = g.add(g.fold(sqd, "add"), g.sq(dy))

    # vd = Je^T d
    vd0 = g.sub(g.mul(A6, dy), g.mul(B6, dx))
    vd1 = g.fold(g.mul(g.rev(A6K6), dxdz), "sub")  # K6*dx - A6*dz
    d62 = g.rev(g.sub(A6K6, A2K2))                 # [K6-K2 | A6-A2]
    vd2 = g.fold(g.mul(d62, dxdz), "sub")
    d63 = g.rev(g.sub(A6K6, EK3))
    vd3 = g.fold(g.mul(d63, dxdz), "sub")
    cd45 = g.mul(c45s45, dxdz)                     # [c45*dx | s45*dz]
    # joint-5 coordinate rescaled by 1/d6: s is invariant when vd4, M[:,4]
    # and M44 are scaled consistently, so the d6 factors fold into constants
    vd4 = g.add(g.fold(cd45, "add"), g.mul(s5, dy))
    vd = [vd0, vd1, vd2, vd3, vd4]

    M = {(0, 0): M00, (0, 1): M01, (0, 2): M02, (0, 3): M03, (0, 4): M04,
         (1, 1): M11, (1, 2): M12, (1, 3): M13, (1, 4): M14,
         (2, 2): M22, (2, 3): M23e, (2, 4): M24,
         (3, 3): M33, (3, 4): M34}

    # Bordered LDL^T on [[M, vd], [vd^T, 0]] (6x6).  No sqrt: the pivot
    # chain (d -> recip -> C/L updates -> d) stays entirely on DVE.  The
    # last pivot d5 = -vd^T M^{-1} vd = -s, so the solve is integrated.
    Mb = dict(M)
    for j in range(5):
        Mb[(j, 5)] = vd[j]
    C = {}   # C[k,j] = L[k,j] * d_j (unnormalized column entries)
    L = {}   # normalized
    r = []
    for jc in range(5):
        if jc == 0:
            dd = Mb[(0, 0)]
        elif jc == 4:
            # M[4,4] is the constant M44C
            dd = g.ts2(g.mul(C[(4, 0)], L[(4, 0)]), -1.0, "mult", M44C, "add")
            for t in range(1, 4):
                dd = g.sub(dd, g.mul(C[(4, t)], L[(4, t)]))
        else:
            dd = Mb[(jc, jc)]
            for t in range(jc):
                dd = g.sub(dd, g.mul(C[(jc, t)], L[(jc, t)]))
        rj = g.recip(dd)
        r.append(rj)
        for kk2 in range(jc + 1, 6):
            a = Mb[(jc, kk2)]
            for t in range(jc):
                a = g.sub(a, g.mul(C[(kk2, t)], L[(jc, t)]))
            C[(kk2, jc)] = a
            L[(kk2, jc)] = g.mul(a, rj)
    # s = sum_t C[5,t]*L[5,t]
    sacc = None
    for t in range(5):
        p = g.mul(C[(5, t)], L[(5, t)])
        sacc = p if sacc is None else g.add(sacc, p)
    cost_neg = g.mul(g.cmul(-1.0, g.recip(sacc)), n2)
    g.finalize_widths()
    return g, cost_neg


# ----------------------------------------------------------------------------
# numpy evaluation of the DAG (for validation in test.py)
# ----------------------------------------------------------------------------

def eval_numpy(g, root, chans):
    """Width-2 node values are tuples (left, right) of arrays."""
    val = {}

    def f32c(v):
        if isinstance(v, tuple):
            return tuple(x.astype(np.float32) for x in v)
        return v.astype(np.float32)

    for n in g.nodes:
        if n.op == "const":
            val[n.id] = np.float32(n.c)
            continue
        if n.op == "in":
            val[n.id] = chans[n.c].astype(np.float32)
            continue
        a = [val[x.id] for x in n.args]
        if n.width >= 2 and n.op not in ("pack", "rev", "half", "vslice",
                                         "fold"):
            w = n.width
            a = [(x,) * w if not isinstance(x, tuple) else x for x in a]

            def bop(f):
                return tuple(f(a[0][i], a[1][i]) for i in range(w))

            def uop(f):
                return tuple(f(a[0][i]) for i in range(w))
        else:
            def bop(f):
                return f(a[0], a[1])

            def uop(f):
                return f(a[0])

        if n.op == "pack":
            v = tuple(a)
        elif n.op == "rev":
            v = (a[0][1], a[0][0])
        elif n.op == "half":
            v = a[0][n.c]
        elif n.op == "vslice":
            v = a[0][n.c[0]:n.c[0] + n.c[1]]
        elif n.op == "fold":
            op, swap = n.c
            l, r = a[0]
            if swap:
                l, r = r, l
            v = (l + r) if op == "add" else (l - r)
        elif n.op == "add":
            v = bop(lambda x, y: x + y)
        elif n.op == "sub":
            v = bop(lambda x, y: x - y)
        elif n.op == "mul":
            v = bop(lambda x, y: x * y)
        elif n.op == "square":
            v = uop(lambda x: x * x)
        elif n.op == "cmul":
            v = uop(lambda x: np.float32(n.c) * x)
        elif n.op == "cadd":
            v = uop(lambda x: x + np.float32(n.c))
        elif n.op == "sin":
            sc, b = n.c
            v = uop(lambda x: np.sin(np.float32(sc) * x + np.float32(b)))
        elif n.op == "ts2":
            s1, op0, s2, op1 = n.c

            def ts2f(x):
                for s_, o_ in ((s1, op0), (s2, op1)):
                    if o_ == "mult":
                        x = x * np.float32(s_)
                    elif o_ == "is_ge":
                        x = (x >= np.float32(s_)).astype(np.float32)
                    else:
                        x = x + np.float32(s_)
                return x
            v = uop(ts2f)
        elif n.op == "sqrt":
            v = uop(np.sqrt)
        elif n.op == "recip":
            v = uop(lambda x: np.float32(1.0) / x)
        else:
            raise ValueError(n.op)
        val[n.id] = f32c(v)
    return val[root.id]


def ref_numpy(x):
    """Full-pipeline numpy reference using the DAG; x [B,H,26] -> [B]."""
    B, H, Cc = x.shape
    N = B * H
    flat = x.reshape(N, Cc).astype(np.float32)
    g, root = build_graph()
    chans = {i: flat[:, SRC_COLS[i]] for i in range(8)}
    cn = eval_numpy(g, root, chans)
    return cn.reshape(B, H).sum(axis=1)


# ----------------------------------------------------------------------------
# planning: STT fusion + ETF list scheduling across dve/act/gps
# ----------------------------------------------------------------------------

# pipelined per-[128,128]-op costs (TimelineSim probe)
COST = {
    ("dve", "tt"): 212.0, ("dve", "stt"): 212.0, ("dve", "ts"): 162.0,
    ("dve", "recip"): 204.0, ("dve", "reduce"): 296.0,
    ("act", "any"): 360.0,
    ("gps", "tt"): 440.0, ("gps", "ts"): 360.0,
}
XLAT = 100.0  # cross-engine semaphore latency


# per-width op costs: measured w=1 base + per-extra-slot slope
def wcost(base, slope, w):
    return base + slope * (w - 1)


def classify(n):
    """Returns options = [(engine, cost), ...]. GPSIMD (Pool) supports only
    tensor_tensor and tensor_scalar; scalar_tensor_tensor is DVE-only."""
    w = n.width
    c_tt = wcost(212.0, 144.0, w)
    c_ts = wcost(162.0, 89.0, w)
    c_act = wcost(360.0, 106.0, w)
    c_gtt = wcost(440.0, 260.0, w)
    c_gts = wcost(360.0, 151.0, w)
    if n.op == "sin" or n.op == "sqrt":
        return [("act", c_act)]
    if n.op == "recip":
        return [("dve", COST[("dve", "recip")])]
    if n.op == "fold":
        return [("dve", COST[("dve", "tt")]), ("gps", COST[("gps", "tt")])]
    if n.op == "square":
        return [("dve", c_tt), ("act", c_act), ("gps", c_gtt)]
    if n.op in ("cadd", "cmul", "ts2"):
        # ts2 with non-(mult,add) pattern can't be an ACT Copy
        actok = True
        if n.op == "ts2" and (n.c[1], n.c[3]) != ("mult", "add"):
            actok = False
        opts = [("dve", c_ts)]
        if actok:
            opts.append(("act", c_act))
        opts.append(("gps", c_gts))
        return opts
    if n.op in ("add", "sub", "mul"):
        if isinstance(n.c, tuple) and n.c and n.c[0] == "stt_cmul":
            return [("dve", c_tt)]
        return [("dve", c_tt), ("gps", c_gtt)]
    if n.op == "cmul_stt":  # cmul fused with mul/square arg
        return [("dve", c_tt)]
    raise ValueError(n.op)


def plan(g, root):
    """STT fusion + ETF scheduling. Returns emit list ordered by virtual
    start time, with n.engine set."""
    # reachability + users
    reach = set()
    stack = [root]
    while stack:
        n = stack.pop()
        if n.id in reach:
            continue
        reach.add(n.id)
        stack.extend(n.args)
    for n in g.nodes:
        n.users = []
    order = [n for n in g.nodes if n.id in reach]
    for n in order:
        for a in n.args:
            a.users.append(n)

    VIEWS = ("pack", "rev", "half", "vslice")

    # fusion: add/sub(x, cmul(c,y)) -> STT ; cmul(c, mul(x,y)/square(x)) -> STT
    # (never fuse away a pack member: its output must land in the pack tile)
    for n in order:
        if n.op in VIEWS:
            continue
        if n.op in ("add", "sub"):
            for k, a in enumerate(n.args):
                if a.op == "cmul" and len(a.users) == 1 and a.fused_into is None \
                        and a.pack_into is None \
                        and a.args[0].fused_into is None \
                        and a.args[0].op != "const":
                    n.c = ("stt_cmul", k, a.c)
                    a.fused_into = n
                    break
        elif n.op == "cmul" and n.fused_into is None:
            a = n.args[0]
            if a.op in ("mul", "square") and len(a.users) == 1 \
                    and a.fused_into is None and a.pack_into is None \
                    and all(aa.fused_into is None for aa in a.args):
                a.fused_into = n

    # effective deps of an emitted node (through fused producers and views)
    def resolve(a, out):
        if a.op in ("const", "in"):
            return
        if a.op == "pack":
            for m in a.args:
                resolve(m, out)
        elif a.op in ("rev", "half", "vslice"):
            resolve(a.args[0], out)
        else:
            out.append(a)

    def deps(n):
        out = []
        for a in n.args:
            if a.fused_into is n:
                for aa in a.args:
                    resolve(aa, out)
            else:
                resolve(a, out)
        return out

    emit_nodes = [n for n in order
                  if n.op not in ("const", "in") and n.op not in VIEWS
                  and n.fused_into is None]

    # ts-class ops occurring after the trig preamble go to the otherwise
    # idle ACT engine (its eligible work is inherently front-loaded)
    max_sin = max((n.id for n in emit_nodes if n.op == "sin"), default=0)

    def opts_of(n):
        if n.op == "cmul" and n.args[0].fused_into is n:
            e = Expr("cmul_stt")
            e.width = n.width
            return classify(e)
        return classify(n)

    # critical-path priority (min cost per node)
    mincost = {n.id: min(c for _, c in opts_of(n)) for n in emit_nodes}
    prio = {}

    def get_prio(n):
        if n.id in prio:
            return prio[n.id]
        p = mincost[n.id] + max(
            (get_prio(u if u.fused_into is None else u.fused_into)
             for u in n.users if (u.fused_into is None or u.fused_into is not n)
             ), default=0.0)
        prio[n.id] = p
        return p

    for n in order:
        n.prio = 0.0
    # prios in reverse topological order; views are zero-cost pass-throughs
    for n in reversed(order):
        if n.op in ("const", "in"):
            continue
        best = 0.0
        for u in n.users:
            tgt = u.fused_into if u.fused_into is not None else u
            if tgt is n:
                continue
            if tgt.op not in ("const", "in"):
                best = max(best, tgt.prio)
        own = mincost[n.id] if n.id in mincost else 0.0
        n.prio = own + best

    # ---- phase 1: static engine assignment (balance max load) ----
    # Critical-chain nodes keep their fastest engine; the rest greedily go to
    # the engine with the smallest resulting load.
    ndeps = {n.id: 0 for n in emit_nodes}
    dep_lists = {}
    for n in emit_nodes:
        dl = deps(n)
        dep_lists[n.id] = dl
        ndeps[n.id] = len(dl)
    users_emit = {n.id: [] for n in emit_nodes}
    for n in emit_nodes:
        for d in dep_lists[n.id]:
            users_emit[d.id].append(n)

    def run_etf(gamma, win, xlat, act_disc=1.0):
        """ETF with load-penalty engine choice. Returns (makespan, sched:
        list of (n, engine, start, finish))."""
        nd = dict(ndeps)
        ready = [n for n in emit_nodes if nd[n.id] == 0]
        eng_free = {"dve": 0.0, "act": 0.0, "gps": 0.0}
        eload = {"dve": 0.0, "act": 0.0, "gps": 0.0}
        fin = {}
        eng_of = {}
        sched = []
        while ready:
            cands = []
            for n in ready:
                dr_cache = {}
                for e, c in opts_of(n):
                    dr = 0.0
                    for d in dep_lists[n.id]:
                        dr = max(dr, fin[d.id] +
                                 (xlat if eng_of[d.id] != e else 0.0))
                    st = max(eng_free[e], dr)
                    ceff = c * act_disc if e == "act" else c
                    score = st + ceff + gamma * eload[e]
                    cands.append((score, n.prio, n, e, c, st))
            smin = min(c[0] for c in cands)
            _, _, n, e, c, st = max(
                (cd for cd in cands if cd[0] <= smin + win),
                key=lambda cd: (cd[1], -cd[0]))
            ready.remove(n)
            fin[n.id] = st + c
            eng_of[n.id] = e
            eng_free[e] = st + c
            eload[e] += c
            sched.append((n, e, st, st + c))
            for u in users_emit[n.id]:
                nd[u.id] -= 1
                if nd[u.id] == 0:
                    ready.append(u)
        return max(f for _, _, _, f in sched), sched

    best_ms, best_sched = None, None
    for gamma in (0.0, 0.02, 0.05, 0.1, 0.2, 0.4, 0.7):
        for win in (0.0, 80.0, 150.0, 250.0):
            for xl in (200.0,):
                for ad in (0.5,):
                    ms, sched = run_etf(gamma, win, xl, ad)
                    if best_ms is None or ms < best_ms:
                        best_ms, best_sched = ms, sched

    load = {"dve": 0.0, "act": 0.0, "gps": 0.0}
    for n, e, st, f in best_sched:
        n.engine = e
        n.start = st
        n.finish = f
        load[e] += f - st

    scheduled = [n for n, _, _, _ in best_sched]
    scheduled.sort(key=lambda n: (n.start, n.finish))
    for i2, n in enumerate(scheduled):
        n.order = i2
    makespan = best_ms
    return scheduled, load, makespan


# ----------------------------------------------------------------------------
# bass emission
# ----------------------------------------------------------------------------

NCORES = 8
B_FULL, H, CH = 2048, 64, 26
N_PER_CORE = B_FULL * H // NCORES          # 16384
P = 128
FD = N_PER_CORE // P                        # 128
NCH = 8


def _build_bass():
    import concourse.bass as bass
    from concourse.bacc import Bacc
    import concourse.mybir as mybir
    from concourse.tile import TileContext

    f32 = mybir.dt.float32
    alu = mybir.AluOpType
    AF = mybir.ActivationFunctionType

    g, root = build_graph()
    emit, load, makespan = plan(g, root)

    nc = Bacc()
    xs = nc.dram_tensor("xs", (P, NCH * FD), f32, kind="ExternalInput")
    out = nc.dram_tensor("out", (B_FULL // NCORES,), f32, kind="ExternalOutput")

    # liveness for slot allocation
    last_use = {}
    for n in emit:
        for a in n.args:
            if a.order is not None:
                last_use[a.id] = max(last_use.get(a.id, -1), n.order)
            if a.fused_into is n:
                for aa in a.args:
                    if aa.order is not None:
                        last_use[aa.id] = max(last_use.get(aa.id, -1), n.order)
    last_use[root.id] = len(emit) + 10

    with TileContext(nc) as tc:
        with tc.tile_pool(name="vals", bufs=1) as vp:
            # three staged input groups: [q2 q3 q4] [q1 q5] [hx hy hz],
            # issued at t=0 on three different HWDGE-capable engines so the
            # fixed DGE latencies overlap; transfers serialize on the DMA bus
            # in issue order (q2/q3/q4 first — head of the trig chain).
            stA = vp.tile([P, 3 * FD], f32, tag="stA", name="stA")
            stB = vp.tile([P, 2 * FD], f32, tag="stB", name="stB")
            stC = vp.tile([P, 3 * FD], f32, tag="stC", name="stC")
            nc.gpsimd.dma_start(stA[:, :], xs[:, 0:3 * FD])
            nc.sync.dma_start(stB[:, :], xs[:, 3 * FD:5 * FD])
            nc.scalar.dma_start(stC[:, :], xs[:, 5 * FD:8 * FD])
            # const APs for non-Copy activation biases (registered after the
            # DMAs so they don't delay them; barrier orders memset vs readers)
            for cv in (PI / 2,):
                t = nc.alloc_sbuf_tensor(f"constf32-{cv}", [128, 1], f32)
                nc.gpsimd.memset(t.ap(), cv)
                nc.const_aps.aps[(f32, float(cv))] = t.ap()
            nc.all_engine_barrier()

            def chan_ap(ch):
                if ch < 3:
                    return stA[:, ch * FD:(ch + 1) * FD]
                if ch < 5:
                    return stB[:, (ch - 3) * FD:(ch - 2) * FD]
                return stC[:, (ch - 5) * FD:(ch - 4) * FD]

            from collections import deque
            free_slots = deque()
            SLACK = 60
            n_slots = [0]
            w_slots = [0]
            node_tile = {}   # id -> (tile, col_lo, ncols)
            pack_tile = {}

            def desc_of(n):
                """(tile, col_lo, ncols) for a value-holding node."""
                if n.op == "pack":
                    if n.id not in pack_tile:
                        t = vp.tile([P, n.width * FD], f32, tag=f"pk{n.id}",
                                    name=f"pk{n.id}", bufs=1)
                        pack_tile[n.id] = t
                    return (pack_tile[n.id], 0, n.width * FD)
                if n.op == "half":
                    t, lo, w = desc_of(n.args[0])
                    return (t, lo + n.c * FD, FD)
                if n.op == "vslice":
                    t, lo, w = desc_of(n.args[0])
                    return (t, lo + n.c[0] * FD, n.c[1] * FD)
                if n.op == "rev":
                    return desc_of(n.args[0])
                return node_tile[n.id]

            def ap2(n):
                if n.op == "in":
                    return chan_ap(n.c)
                t, lo, w = desc_of(n)
                return t[:, lo:lo + w]

            def apw(n, w):
                """[P, w, FD] view: wide node, reversed pair, or broadcast."""
                if n.op == "rev":
                    return apw(n.args[0], 2)[:, ::-1, :]
                if n.width == w:
                    return ap2(n).rearrange("p (c q) -> p c q", c=w)
                assert n.width == 1
                return ap2(n).unsqueeze(1).broadcast_to([P, w, FD])

            def alloc(n):
                if n.pack_into is not None:
                    pk, slot = n.pack_into
                    t, lo, w = desc_of(pk)
                    node_tile[n.id] = (t, lo + slot * FD, FD)
                    return t[:, slot * FD:(slot + 1) * FD]
                if n.width > 1:
                    sl = w_slots[0]
                    w_slots[0] += 1
                    t = vp.tile([P, n.width * FD], f32, tag=f"w{sl}x{n.width}",
                                name=f"v{n.id}", bufs=1)
                    node_tile[n.id] = (t, 0, n.width * FD)
                    return t[:, :]
                if len(free_slots) > SLACK:
                    sl = free_slots.popleft()
                else:
                    sl = n_slots[0]
                    n_slots[0] += 1
                t = vp.tile([P, FD], f32, tag=f"s{sl}", name=f"v{n.id}", bufs=2)
                n.slot = sl
                node_tile[n.id] = (t, 0, FD)
                return t[:, :]

            by_last = {}
            for nid, lu in last_use.items():
                by_last.setdefault(lu, []).append(nid)

            eng = {"dve": nc.vector, "act": nc.scalar, "gps": nc.gpsimd}
            ALU_OF = {"add": alu.add, "sub": alu.subtract, "mul": alu.mult}

            def needs3(n, tens_args):
                if n.width < 2:
                    return False
                return any(a.op == "rev" or a.width != n.width
                           for a in tens_args if a.op not in ("const",))

            def aps_for(n, ot2, tens_args):
                """Return (out_ap, [arg_aps]) with matching dimensionality."""
                if needs3(n, tens_args):
                    o3 = ot2.rearrange("p (c q) -> p c q", c=n.width)
                    return o3, [apw(a, n.width) for a in tens_args]
                return ot2, [ap2(a) for a in tens_args]

            for n in emit:
                ot = alloc(n)
                e = eng[n.engine]
                en = n.engine
                if n.op == "sin":
                    sc, b = n.c
                    nc.scalar.activation(ot, ap2(n.args[0]), AF.Sin,
                                         bias=float(b), scale=float(sc))
                elif n.op == "sqrt":
                    nc.scalar.activation(ot, ap2(n.args[0]), AF.Sqrt)
                elif n.op == "recip":
                    nc.vector.reciprocal_approx_fast(out=ot, in_=ap2(n.args[0]))
                elif n.op == "fold":
                    fop, swap = n.c
                    t, lo, w = desc_of(n.args[0])
                    l = t[:, lo:lo + FD]
                    r = t[:, lo + FD:lo + 2 * FD]
                    if swap:
                        l, r = r, l
                    e.tensor_tensor(ot, l, r,
                                    alu.add if fop == "add" else alu.subtract)
                elif n.op == "square":
                    oa, (ia,) = aps_for(n, ot, [n.args[0]])
                    if en == "act":
                        nc.scalar.activation(oa, ia, AF.Square)
                    else:
                        e.tensor_tensor(oa, ia, ia, alu.mult)
                elif n.op == "cadd":
                    oa, (ia,) = aps_for(n, ot, [n.args[0]])
                    if en == "act":
                        nc.scalar.activation(oa, ia, AF.Copy,
                                             bias=float(n.c), scale=1.0)
                    else:
                        e.tensor_scalar_add(oa, ia, float(n.c))
                elif n.op == "ts2":
                    s1, op0, s2, op1 = n.c
                    oa, (ia,) = aps_for(n, ot, [n.args[0]])
                    if en == "act":
                        nc.scalar.activation(oa, ia, AF.Copy,
                                             bias=float(s2), scale=float(s1))
                    else:
                        e.tensor_scalar(oa, ia, float(s1), float(s2),
                                        getattr(alu, op0), getattr(alu, op1))
                elif n.op == "cmul":
                    a = n.args[0]
                    if a.fused_into is n:
                        if a.op == "square":
                            x = yv = a.args[0]
                        else:
                            x, yv = a.args
                        oa, (xa, ya) = aps_for(n, ot, [x, yv])
                        e.scalar_tensor_tensor(oa, xa, float(n.c),
                                               ya, alu.mult, alu.mult)
                    else:
                        oa, (ia,) = aps_for(n, ot, [a])
                        if en == "act":
                            nc.scalar.activation(oa, ia, AF.Copy,
                                                 bias=0.0, scale=float(n.c))
                        else:
                            e.tensor_scalar_mul(oa, ia, float(n.c))
                elif n.op in ("add", "sub"):
                    if isinstance(n.c, tuple) and n.c and n.c[0] == "stt_cmul":
                        _, k, cval = n.c
                        cm = n.args[k]
                        other = n.args[1 - k]
                        x = cm.args[0]
                        oa, (xa, ya) = aps_for(n, ot, [x, other])
                        if n.op == "add":
                            e.scalar_tensor_tensor(oa, xa, float(cval),
                                                   ya, alu.mult, alu.add)
                        elif k == 1:
                            e.scalar_tensor_tensor(oa, xa, float(-cval),
                                                   ya, alu.mult, alu.add)
                        else:
                            e.scalar_tensor_tensor(oa, xa, float(cval),
                                                   ya, alu.mult, alu.subtract)
                    else:
                        oa, (xa, ya) = aps_for(n, ot, [n.args[0], n.args[1]])
                        e.tensor_tensor(oa, xa, ya, ALU_OF[n.op])
                elif n.op == "mul":
                    oa, (xa, ya) = aps_for(n, ot, [n.args[0], n.args[1]])
                    e.tensor_tensor(oa, xa, ya, alu.mult)
                else:
                    raise ValueError(n.op)

                for nid in by_last.get(n.order, []):
                    nd = g.nodes[nid]
                    if nd.slot is not None and nd.id != root.id:
                        free_slots.append(nd.slot)
                        nd.slot = None

            # epilogue: per-b sums (64-sample segments)
            osum = vp.tile([P, 2], f32, tag="osum", bufs=2)
            ct, clo, _cw = node_tile[root.id]
            cr3 = ct[:, clo:clo + 128].rearrange("p (j q) -> p j q", j=2)
            nc.vector.tensor_reduce(osum[:, 0:2].unsqueeze(2), cr3,
                                    mybir.AxisListType.X, alu.add)
            nc.gpsimd.dma_start(out.rearrange("(p j) -> p j", p=P), osum[:, :])

    nc.compile()
    return nc, len(emit), load, makespan


_CACHE = {}


def kernel(x, cond, time):
    from concourse.bass_utils import run_bass_kernel_spmd

    if "nc" not in _CACHE:
        nc, n_ops, load, makespan = _build_bass()
        _CACHE["nc"] = nc
        _CACHE["stats"] = (n_ops, load, makespan)
    nc = _CACHE["nc"]

    xf = np.ascontiguousarray(x, dtype=np.float32).reshape(B_FULL * H, CH)
    sel = xf[:, SRC_COLS]                                   # [131072, 8]
    in_maps = []
    for k in range(NCORES):
        shard = sel[k * N_PER_CORE:(k + 1) * N_PER_CORE]    # [16384, 8]
        # [128 p, 128 q, 8 c] -> [128, 8, 128] channel-major free layout
        arr = shard.reshape(P, FD, NCH).transpose(0, 2, 1)
        in_maps.append({"xs": np.ascontiguousarray(arr).reshape(P, NCH * FD)})
    res = run_bass_kernel_spmd(nc, in_maps, core_ids=list(range(NCORES)))
    _CACHE["exec_time_ns"] = res.exec_time_ns
    _CACHE["trace"] = res.instructions_and_trace
    outs = [res.results[k]["out"] for k in range(NCORES)]
    return np.concatenate(outs).astype(np.float32)


if __name__ == "__main__":
    g, root = build_graph()
    emit, load, makespan = plan(g, root)
    from collections import Counter
    print("emitted ops:", len(emit))
    print(Counter((n.engine, n.op) for n in emit))
    print("load est (us):", {k: v / 1000 for k, v in load.items()})
    print("virtual makespan (us):", makespan / 1000)
